# revision 12
# baseline (speedup 1.0000x reference)
"""Trainium2 Bass kernel for BitNet-style cross-attention (8 NeuronCores).

Data-parallel token sharding: b=2, n=2048 -> 4096 query-token rows; each of
the 8 cores owns 512 (cores 0-3 batch 0, 4-7 batch 1) and computes its output
slice independently (k/v recomputed per core).

v3: streamed attention.  The kernel runs in four overlapped phases:
  A/B: x quant + Q proj; ctx half 0 quant + K/V proj; wq/wv/wk quant.
  C:   attention over ctx half 0 (scores+exp+attn@v, po accumulated in
       PSUM then parked unnormalized in SBUF), with ctx-half-1 quant +
       K/V projection emitted as PE/DVE "filler" between attention slabs
       so the Act-engine exp stream (the largest fixed cost) hides under
       projection work.
  D:   attention over ctx half 1 (po += half-1, then per-head softmax
       normalize straight from PSUM), with wo quant as filler.
  tail: out act-quant + output projection.

Quant path: per-token absmax comes from a natural-layout (token-major) copy
of x/ctx (contiguous free-axis reduce) so per-token scales are [P,1] columns
(icT/esc/vsc need no transposes).  Scale rows are broadcast across
partitions once per 512-token group (tiny PE transpose MMs + one gpsimd
partition_broadcast) and the feature-major round chain is 2 fused DVE ops
per eighth via stride-0 broadcast APs.  round() uses the fp32
magic-constant trick.  Softmax denominators accumulate via an extra ones
column in v; their reciprocals use the fast approx DVE reciprocal.
"""

import numpy as np

import concourse.bass as bass
import concourse.mybir as mybir
import concourse.tile as tile
from concourse import bacc, bass_isa
from concourse.bass_utils import run_bass_kernel_spmd

F32 = mybir.dt.float32
BF16 = mybir.dt.bfloat16
AX = mybir.AxisListType
OP = mybir.AluOpType
AF = mybir.ActivationFunctionType

P = 128
MAGIC = 12582912.0  # 1.5 * 2**23: fp32 add/sub rounds to nearest int (ties even)

CFG_FULL = dict(DIM=1024, INNER=1024, H=16, D=64, NTOK=512, MCTX=2048)
N_CORES = 8
EPS = 1e-5
APPROX_DENS = False  # approx reciprocal for softmax denominators


def build(cfg):
    DIM, INNER, H, D = cfg["DIM"], cfg["INNER"], cfg["H"], cfg["D"]
    NTOK, MCTX = cfg["NTOK"], cfg["MCTX"]
    KC = DIM // P            # input-dim 128-chunks (8)
    IC = INNER // P          # inner-dim 128-chunks (8)
    NKB = MCTX // P          # ctx 128-blocks (16)
    NTB = NTOK // P          # query-token 128-blocks (4)
    QTOK = 512               # K-proj moving width
    ETOK = 256               # ctx staging eighth size
    NE = MCTX // ETOK        # 8 eighths
    EKB = ETOK // P          # ctx 128-blocks per eighth (2)
    HKB = NKB // 2           # ctx 128-blocks per half (8)
    VW = D + 1               # v columns per head incl ones
    HPH = (INNER // 2) // D  # heads per inner half (8)
    NP = H // 2              # head pairs (8)

    nc = bacc.Bacc("TRN2", target_bir_lowering=False, debug=False,
                   num_devices=N_CORES)

    xT = nc.dram_tensor("xT", [DIM, NTOK], F32, kind="ExternalInput")
    xN = nc.dram_tensor("xN", [NTOK, DIM], F32, kind="ExternalInput")
    cT = nc.dram_tensor("cT", [DIM, MCTX], F32, kind="ExternalInput")
    cN = nc.dram_tensor("cN", [MCTX, DIM], F32, kind="ExternalInput")
    wT = {}
    for w in ("wq", "wk", "wv", "wo"):
        wT[w] = nc.dram_tensor(w + "T", [DIM, INNER], F32, kind="ExternalInput")
    iden = nc.dram_tensor("iden", [P, P], F32, kind="ExternalInput")
    y_out = nc.dram_tensor("y", [NTOK, DIM], F32, kind="ExternalOutput")

    from contextlib import ExitStack
    with tile.TileContext(nc) as tc, ExitStack() as ctx:
        # ---- long-lived pools -------------------------------------------
        pp = ctx.enter_context(tc.tile_pool(name="persist", bufs=1))
        smp = ctx.enter_context(tc.tile_pool(name="small", bufs=1))
        asp = ctx.enter_context(tc.tile_pool(name="astage", bufs=2))
        # PSUM: ps_mm [*,512] 1-bank tiles (pq/pk/pv/bcast/py) = 2 banks;
        # ps_ss (scores) 2 banks + ps_po 4 banks opened for the attention
        # phases; ps_y for the tail after those close.  Max live = 8 banks.
        ps_mm = ctx.enter_context(tc.tile_pool(name="ps_mm", bufs=2,
                                               space="PSUM"))

        qb = pp.tile([P, IC, NTOK], BF16, tag="qb")     # q*inv_x, feat-major
        kb = pp.tile([P, IC, MCTX], BF16, tag="kb")     # k raw ints, feat-major
        vb = pp.tile([P, NKB * H * VW], BF16, tag="vb")  # v natural + ones col
        vb3 = vb[:].rearrange("p (k h w) -> p k h w", h=H, w=VW)
        idt = pp.tile([P, P], F32, tag="idt")           # identity for PE transp
        nc.sync.dma_start(out=idt[:], in_=iden.ap()[:, :])
        icT = pp.tile([P, NKB], F32, tag="icT")         # inv_c, ctx-token-major
        rqcT = pp.tile([P, NKB], F32, tag="rqcT")       # 127/absmax_c tok-major
        vsc = pp.tile([P, NKB], F32, tag="vsc")         # icT * mean|wv|
        esc = pp.tile([P, NKB], F32, tag="esc")         # icT * mq*mk/sqrt(D)

        wmean = {}

        # ---- weight quantization ----------------------------------------
        def quant_weight(w, wsp, dst_pool, tern_eng="act"):
            NST = 2
            HCH = KC // NST
            HW = HCH * INNER
            wbt = dst_pool.tile([P, KC * INNER], BF16, tag="wb_" + w,
                                name="wb_" + w)
            wsums = smp.tile([P, NST], F32, tag="wsum2")
            stages = []
            for hh in range(NST):
                ws = wsp.tile([P, HW], F32, tag="wst", name=f"ws_{w}{hh}")
                for c in range(HCH):
                    cc = hh * HCH + c
                    nc.sync.dma_start(
                        out=ws[:, c * INNER:(c + 1) * INNER],
                        in_=wT[w].ap()[cc * P:(cc + 1) * P, :])
                nc.scalar.activation(wbt[:, hh * HW:(hh + 1) * HW], ws[:],
                                     AF.Abs, accum_out=wsums[:, hh:hh + 1])
                stages.append(ws)
            wsum = smp.tile([P, 1], F32, tag="wsum")
            nc.vector.tensor_reduce(wsum[:], wsums[:], axis=AX.X, op=OP.add)
            wrep = smp.tile([P, 1], F32, tag="wrep")
            nc.gpsimd.partition_all_reduce(wrep[:], wsum[:], channels=P,
                                           reduce_op=bass_isa.ReduceOp.add)
            mean = smp.tile([P, 1], F32, tag="wmean_" + w, name="mean_" + w)
            nc.vector.tensor_scalar(mean[:], wrep[:], 1.0 / (DIM * INNER),
                                    EPS, OP.mult, OP.max)
            qs = smp.tile([P, 1], F32, tag="wqs_" + w, name="qs_" + w)
            nc.vector.reciprocal(qs[:], mean[:])
            wmean[w] = mean
            for hh, ws in enumerate(stages):
                nc.vector.tensor_scalar(ws[:], ws[:], qs[:], 1.49,
                                        OP.mult, OP.min)
                nc.vector.tensor_scalar(ws[:], ws[:], -1.49, MAGIC,
                                        OP.max, OP.add)
                if tern_eng == "act":
                    nc.scalar.activation(wbt[:, hh * HW:(hh + 1) * HW],
                                         ws[:], AF.Copy, bias=-MAGIC)
                else:
                    nc.vector.tensor_scalar(wbt[:, hh * HW:(hh + 1) * HW],
                                            ws[:], -MAGIC, None, OP.add)
            return wbt

        # wo variant: quarter-staged with re-DMA (small SBUF footprint, runs
        # as filler during attention half 1).  Emitted in units so it can be
        # interleaved between attention slabs.
        def quant_weight_wo_units(wsp, dst_pool):
            w = "wo"
            NWQ = 4
            CPQ = KC // NWQ
            WQW = CPQ * INNER
            wbt = dst_pool.tile([P, KC * INNER], BF16, tag="wb_wo",
                                name="wb_wo")
            wsums = smp.tile([P, NWQ], F32, tag="wsums")
            units = []

            def u_a(i):
                ws = wsp.tile([P, WQW], F32, tag="wstq", name=f"woA{i}")
                for c in range(CPQ):
                    cc = i * CPQ + c
                    nc.sync.dma_start(out=ws[:, c * INNER:(c + 1) * INNER],
                                      in_=wT[w].ap()[cc * P:(cc + 1) * P, :])
                nc.vector.tensor_reduce(wsums[:, i:i + 1], ws[:],
                                        axis=AX.X, op=OP.add,
                                        apply_absolute_value=True)

            def u_mid():
                wsum = smp.tile([P, 1], F32, tag="wsum")
                nc.vector.tensor_reduce(wsum[:], wsums[:], axis=AX.X,
                                        op=OP.add)
                wrep = smp.tile([P, 1], F32, tag="wrep")
                nc.gpsimd.partition_all_reduce(
                    wrep[:], wsum[:], channels=P,
                    reduce_op=bass_isa.ReduceOp.add)
                mean = smp.tile([P, 1], F32, tag="wmean_wo", name="mean_wo")
                nc.vector.tensor_scalar(mean[:], wrep[:],
                                        1.0 / (DIM * INNER), EPS,
                                        OP.mult, OP.max)
                qs = smp.tile([P, 1], F32, tag="wqs_wo", name="qs_wo")
                nc.vector.reciprocal(qs[:], mean[:])
                wmean[w] = mean
                smp_ref["qs_wo"] = qs

            def u_b(i):
                qs = smp_ref["qs_wo"]
                ws = wsp.tile([P, WQW], F32, tag="wstq", name=f"woB{i}")
                for c in range(CPQ):
                    cc = i * CPQ + c
                    nc.sync.dma_start(out=ws[:, c * INNER:(c + 1) * INNER],
                                      in_=wT[w].ap()[cc * P:(cc + 1) * P, :])
                nc.vector.tensor_scalar(ws[:], ws[:], qs[:], 1.49,
                                        OP.mult, OP.min)
                nc.vector.tensor_scalar(ws[:], ws[:], -1.49, MAGIC,
                                        OP.max, OP.add)
                nc.vector.tensor_scalar(wbt[:, i * WQW:(i + 1) * WQW],
                                        ws[:], -MAGIC, None, OP.add)

            smp_ref = {}
            for i in range(NWQ):
                units.append(lambda i=i: u_a(i))
            units.append(u_mid)
            for i in range(NWQ):
                units.append(lambda i=i: u_b(i))
            return wbt, units

        # ---- token-major scale machinery --------------------------------
        def blk_amax(src_dram, blk, inv_col, rq_col, name):
            bn = cnp.tile([P, DIM], F32, tag="bn", name=name)
            nc.sync.dma_start(out=bn[:],
                              in_=src_dram.ap()[blk * P:(blk + 1) * P, :])
            am = smp.tile([P, 1], F32, tag="bam", name="am_" + name)
            nc.vector.tensor_reduce(am[:], bn[:], axis=AX.X, op=OP.max,
                                    apply_absolute_value=True)
            nc.vector.tensor_scalar(inv_col, am[:], EPS, 1.0 / 127.0,
                                    OP.max, OP.mult)
            nc.vector.reciprocal(rq_col, inv_col)

        # transpose nb [P,1] scale columns into one [1, nb*128] row via tiny
        # PE matmuls, evict, broadcast across partitions on gpsimd.
        def bcast_cols(cols_ap, nb, name):
            pt = ps_mm.tile([1, nb * P], F32, tag="psmm", name="bc_" + name)
            for b in range(nb):
                nc.tensor.matmul(pt[0:1, b * P:(b + 1) * P],
                                 cols_ap[:, b:b + 1], idt[:],
                                 start=True, stop=True)
            row = asp.tile([1, nb * P], F32, tag="bcrow", name="bcr_" + name)
            nc.vector.tensor_copy(row[:], pt[:])
            big = asp.tile([P, nb * P], F32, tag="bcbig", name="bcb_" + name)
            nc.gpsimd.partition_broadcast(big[:], row[:])
            return big

        # fused round chain: dst = round(src * rq) over [P, KC, ntok]
        def round_chunks(dst_ap, src_ap, rq_big_ap, ntok):
            rqb = rq_big_ap.unsqueeze(1).to_broadcast((P, KC, ntok))
            nc.vector.tensor_tensor(src_ap, src_ap, rqb, op=OP.mult)
            nc.vector.tensor_scalar(dst_ap, src_ap, MAGIC, -MAGIC,
                                    OP.add, OP.add)

        def ctx_dma(e):
            cs = csp.tile([P, KC, ETOK], F32, tag="cs", name=f"cs{e}")
            col0 = e * ETOK
            for c in range(KC):
                nc.sync.dma_start(
                    out=cs[:, c, :],
                    in_=cT.ap()[c * P:(c + 1) * P, col0:col0 + ETOK])
            return cs

        def ctx_amax(e):
            for b in range(EKB):
                kbk = e * EKB + b
                blk_amax(cN, kbk, icT[:, kbk:kbk + 1],
                         rqcT[:, kbk:kbk + 1], f"cn{kbk}")

        def scales_for_eighth(e, qkm):
            sl = slice(e * EKB, (e + 1) * EKB)
            nc.vector.tensor_scalar(vsc[:, sl], icT[:, sl],
                                    wmean["wv"][:], None, OP.mult)
            nc.vector.tensor_scalar(esc[:, sl], icT[:, sl], qkm[:], None,
                                    OP.mult)

        def ctx_round(e, cs, cdq, rq_big):
            lcol = (e % (NE // 2)) * ETOK
            half = (e % 2) * ETOK
            round_chunks(cdq[:, :, lcol:lcol + ETOK], cs[:],
                         rq_big[:, half:half + ETOK], ETOK)

        # one K-proj chunk: project `ics` inner chunks for q-quarter q
        def k_proj(q, wkb3, cdq, ics, evict):
            lcol = (q % 2) * QTOK
            for ic in ics:
                pk = ps_mm.tile([P, QTOK], F32, tag="psmm",
                                name=f"pk{q}_{ic}")
                for c in range(KC):
                    nc.tensor.matmul(
                        pk[:], wkb3[:, c, ic * P:(ic + 1) * P],
                        cdq[:, c, lcol:lcol + QTOK],
                        start=(c == 0), stop=(c == KC - 1))
                dst = kb[:, ic, q * QTOK:(q + 1) * QTOK]
                if evict == "act":
                    nc.scalar.copy(dst, pk[:])
                else:
                    nc.vector.tensor_copy(dst, pk[:])

        # one V-proj chunk: ctx block kbk, inner half ih
        def v_proj(kbk, ih, wvb3, cdq, evict):
            kk = kbk % HKB
            pv = ps_mm.tile([P, INNER // 2], F32, tag="psmm",
                            name=f"pv{kbk}_{ih}")
            for c in range(KC):
                nc.tensor.matmul(
                    pv[:],
                    cdq[:, c, kk * P:(kk + 1) * P],
                    wvb3[:, c, ih * (INNER // 2):(ih + 1) * (INNER // 2)],
                    start=(c == 0), stop=(c == KC - 1))
            dst = vb3[:, kbk, ih * HPH:(ih + 1) * HPH, 0:D]
            src = pv[:].rearrange("p (h d) -> p h d", d=D)
            if evict == "act":
                nc.scalar.mul(dst, src, vsc[:, kbk:kbk + 1])
            else:
                nc.vector.tensor_scalar(dst, src, vsc[:, kbk:kbk + 1],
                                        None, OP.mult)

        # ---- attention --------------------------------------------------
        def attn_half(h, fillers, state):
            fi = iter(fillers)

            def fill():
                try:
                    f = next(fi)
                except StopIteration:
                    return
                if f is not None:
                    f()

            for hp in range(NP):
                hA, hB = 2 * hp, 2 * hp + 1
                pA, pB = (hA * D) % P, (hB * D) % P
                po = [ps_po.tile([VW, NTOK], F32, tag="po",
                                 name=f"po{h}_{hp}_{j}") for j in range(2)]
                for i in range(HKB):
                    kbk = h * HKB + i
                    ss = ps_ss.tile([P, 2, NTOK], F32, tag="ss")
                    for j, ph in enumerate((pA, pB)):
                        nc.tensor.matmul(
                            ss[:, j, :],
                            kb[ph:ph + D, hp, kbk * P:(kbk + 1) * P],
                            qb[ph:ph + D, hp, :],
                            start=True, stop=True)
                    et = ep.tile([P, 2, NTOK], BF16, tag="et")
                    nc.scalar.activation(et[:], ss[:], AF.Exp,
                                         scale=esc[:, kbk:kbk + 1])
                    fill()
                    for j, hh in enumerate((hA, hB)):
                        nc.tensor.matmul(
                            po[j][0:VW, :],
                            vb3[:, kbk, hh, :],
                            et[:, j, :],
                            start=(i == 0), stop=(i == HKB - 1))
                if h == 0:
                    for j in range(2):
                        nc.vector.tensor_copy(otU[0:VW, 2 * hp + j, :],
                                              po[j][:])
                else:
                    dens = rbs.tile([1, 2, NTOK], F32, tag="dens",
                                    name=f"dens{hp}")
                    accs = []
                    for j in range(2):
                        acc = rbp.tile([VW, NTOK], F32, tag=f"acc{j}",
                                       name=f"acc{hp}_{j}")
                        nc.vector.scalar_tensor_tensor(
                            acc[:], po[j][:], 0.0,
                            otU[0:VW, 2 * hp + j, :], OP.add, OP.add)
                        nc.vector.tensor_copy(dens[0:1, j, :],
                                              acc[D:D + 1, :])
                        accs.append(acc)
                    rdp = rbs.tile([1, 2, NTOK], F32, tag="rdp",
                                   name=f"rdp{hp}")
                    if APPROX_DENS:
                        nc.vector.reciprocal_approx_fast(rdp[:], dens[:])
                    else:
                        nc.vector.reciprocal(rdp[:], dens[:])
                    for j, (hh, ph) in enumerate([(hA, pA), (hB, pB)]):
                        rb = rbs.tile([D, NTOK], F32, tag="rb")
                        nc.gpsimd.partition_broadcast(rb[:], rdp[0:1, j, :])
                        nc.vector.tensor_tensor(otT[ph:ph + D, hp, :],
                                                accs[j][0:D, :], rb[:],
                                                op=OP.mult)
                    orep = rbs.tile([P, NTOK], F32, tag="orep",
                                    name=f"orep{hp}")
                    nc.gpsimd.partition_all_reduce(
                        orep[:], otT[:, hp, :], channels=P,
                        reduce_op=bass_isa.ReduceOp.absmax)
                    if hp == 0:
                        nc.vector.tensor_copy(oam[:], orep[:])
                    else:
                        nc.vector.tensor_tensor(oam[:], oam[:], orep[:],
                                                op=OP.max)
            # drain leftover fillers
            for f in fi:
                if f is not None:
                    f()

        # ================= emission =====================================
        # Left-stack pool lifetimes: csp/cnp (staging, close end of C),
        # cq0 (ctx half-0 ints, closes end of B), wstage (closes end of B,
        # opened after cq0), xstage/xq/wbqq (A only), cq1 (C only).
        csp_cm = tc.tile_pool(name="cstage", bufs=2)
        csp = csp_cm.__enter__()
        cnp_cm = tc.tile_pool(name="cnstage", bufs=2)
        cnp = cnp_cm.__enter__()
        cq0_cm = tc.tile_pool(name="cq0", bufs=1)
        cq0p = cq0_cm.__enter__()
        cdq0 = cq0p.tile([P, KC, MCTX // 2], BF16, tag="cdq0")
        wsp_cm = tc.tile_pool(name="wstage", bufs=2)
        wsp = wsp_cm.__enter__()

        # ---------- phase A: x + wq + Q proj ----------
        with tc.tile_pool(name="xq", bufs=1) as xqp:
            with tc.tile_pool(name="xstage", bufs=1) as xsp:
                xs = xsp.tile([P, KC, NTOK], F32, tag="xs")
                for c in range(KC):
                    nc.sync.dma_start(
                        out=xs[:, c, :],
                        in_=xT.ap()[c * P:(c + 1) * P, :])
                invx4 = smp.tile([P, NTB], F32, tag="invx4")
                rqx4 = smp.tile([P, NTB], F32, tag="rqx4")
                for b in range(NTB):
                    blk_amax(xN, b, invx4[:, b:b + 1], rqx4[:, b:b + 1],
                             f"xn{b}")
                cs0 = ctx_dma(0)
                ctx_amax(0)
                cs1 = ctx_dma(1)
                ctx_amax(1)
                rqx_b = bcast_cols(rqx4[:], NTB, "rqx")
                invx_b = bcast_cols(invx4[:], NTB, "invx")
                xdq = xqp.tile([P, KC, NTOK], BF16, tag="xdq")
                round_chunks(xdq[:, :, 0:ETOK], xs[:, :, 0:ETOK],
                             rqx_b[:, 0:ETOK], ETOK)
                round_chunks(xdq[:, :, ETOK:NTOK], xs[:, :, ETOK:NTOK],
                             rqx_b[:, ETOK:NTOK], ETOK)

            with tc.tile_pool(name="wbqq", bufs=1) as wbpq:
                wqb = quant_weight("wq", wsp, wbpq, tern_eng="act")
                rqc_b0 = bcast_cols(rqcT[:, 0:NTB], NTB, "rqc0")
                ctx_round(0, cs0, cdq0, rqc_b0)
                ctx_round(1, cs1, cdq0, rqc_b0)
                wqb3 = wqb[:].rearrange("p (c i) -> p c i", c=KC)
                for ic in range(IC):
                    pq = ps_mm.tile([P, NTOK], F32, tag="psmm",
                                    name=f"pq{ic}")
                    for c in range(KC):
                        nc.tensor.matmul(
                            pq[:], wqb3[:, c, ic * P:(ic + 1) * P],
                            xdq[:, c, :],
                            start=(c == 0), stop=(c == KC - 1))
                    nc.vector.tensor_tensor(qb[:, ic, :], pq[:],
                                            invx_b[:], op=OP.mult)

        # ---------- phase B: ctx half 0 + wv/wk + K/V proj half 0 --------
        cs2 = ctx_dma(2)
        ctx_amax(2)
        cs3 = ctx_dma(3)
        ctx_amax(3)
        rqc_b1 = bcast_cols(rqcT[:, NTB:2 * NTB], NTB, "rqc1")
        ctx_round(2, cs2, cdq0, rqc_b1)
        ctx_round(3, cs3, cdq0, rqc_b1)
        wbpv = ctx.enter_context(
            tc.tile_pool(name="wbqv", bufs=1, side="right"))
        wvb = quant_weight("wv", wsp, wbpv, tern_eng="act")
        wvb3 = wvb[:].rearrange("p (c i) -> p c i", c=KC)
        wbpk = ctx.enter_context(
            tc.tile_pool(name="wbqk", bufs=1, side="right"))
        wkb = quant_weight("wk", wsp, wbpk, tern_eng="act")
        wkb3 = wkb[:].rearrange("p (c i) -> p c i", c=KC)

        qkm = smp.tile([P, 1], F32, tag="qkm")
        nc.vector.tensor_tensor(qkm[:], wmean["wq"][:], wmean["wk"][:],
                                op=OP.mult)
        nc.vector.tensor_scalar(qkm[:], qkm[:], 1.0 / float(np.sqrt(D)),
                                None, OP.mult)
        for e in range(4):
            scales_for_eighth(e, qkm)
        nc.vector.memset(vb3[:, :, :, D], 1.0)

        k_proj(0, wkb3, cdq0, range(IC), evict="act")
        k_proj(1, wkb3, cdq0, range(IC), evict="act")
        for kbk in range(HKB):
            for ih in range(2):
                v_proj(kbk, ih, wvb3, cdq0, evict="act")

        wsp_cm.__exit__(None, None, None)
        cq0_cm.__exit__(None, None, None)

        # ---------- phase C: attention half 0 | ctx half 1 + K/V ---------
        cq1_cm = tc.tile_pool(name="cq1", bufs=1)
        cq1p = cq1_cm.__enter__()
        cdq1 = cq1p.tile([P, KC, MCTX // 2], BF16, tag="cdq1")

        otup = ctx.enter_context(tc.tile_pool(name="otup", bufs=1,
                                              side="right"))
        otU = otup.tile([VW, H, NTOK], BF16, tag="otU")
        ep = ctx.enter_context(tc.tile_pool(name="etile", bufs=2,
                                            side="right"))
        rbp = ctx.enter_context(tc.tile_pool(name="rbpool", bufs=2,
                                             side="right"))
        rbs = ctx.enter_context(tc.tile_pool(name="rbsing", bufs=1,
                                             side="right"))
        op_pool = ctx.enter_context(tc.tile_pool(name="opool", bufs=1,
                                                 side="right"))
        otT = op_pool.tile([P, IC, NTOK], BF16, tag="otT")
        oam = op_pool.tile([P, NTOK], F32, tag="oam")

        ps_ss_cm = tc.tile_pool(name="ps_ss", bufs=1, space="PSUM")
        ps_ss = ps_ss_cm.__enter__()
        ps_po_cm = tc.tile_pool(name="ps_po", bufs=4, space="PSUM")
        ps_po = ps_po_cm.__enter__()

        st = {}

        def u_cdma(e):
            st[e] = ctx_dma(e)

        def u_amax(e):
            ctx_amax(e)
            scales_for_eighth(e, qkm)

        def u_bcast(q):
            st["rq%d" % q] = bcast_cols(rqcT[:, q * NTB:(q + 1) * NTB],
                                        NTB, f"rqc{q}")

        def u_round(e):
            ctx_round(e, st.pop(e), cdq1, st["rq%d" % (e // 2)])

        fillers = []

        def F(fn, *a):
            fillers.append(lambda fn=fn, a=a: fn(*a))

        F(u_cdma, 4)
        F(u_amax, 4)
        F(u_cdma, 5)
        F(u_amax, 5)
        F(u_bcast, 2)
        F(u_round, 4)
        F(u_round, 5)
        for ic0 in range(0, IC, 2):
            F(k_proj, 2, wkb3, cdq1, [ic0, ic0 + 1], "dve")
        F(u_cdma, 6)
        F(u_amax, 6)
        F(u_cdma, 7)
        F(u_amax, 7)
        F(u_bcast, 3)
        F(u_round, 6)
        F(u_round, 7)
        for ic0 in range(0, IC, 2):
            F(k_proj, 3, wkb3, cdq1, [ic0, ic0 + 1], "dve")
        for kbk in range(HKB, NKB):
            for ih in range(2):
                F(v_proj, kbk, ih, wvb3, cdq1, "dve")

        attn_half(0, fillers, st)

        cq1_cm.__exit__(None, None, None)
        cnp_cm.__exit__(None, None, None)
        csp_cm.__exit__(None, None, None)

        # ---------- phase D: attention half 1 | wo quant -----------------
        wop = ctx.enter_context(tc.tile_pool(name="wopool", bufs=1,
                                             side="right"))
        wsp2_cm = tc.tile_pool(name="wstage2", bufs=2)
        wsp2 = wsp2_cm.__enter__()
        wob, wo_units = quant_weight_wo_units(wsp2, wop)

        attn_half(1, wo_units, st)

        wsp2_cm.__exit__(None, None, None)
        ps_po_cm.__exit__(None, None, None)
        ps_ss_cm.__exit__(None, None, None)

        # ---------- tail: attn-out quantization + output projection ------
        with tc.tile_pool(name="oq", bufs=2) as oqp, \
                tc.tile_pool(name="ysb", bufs=4) as yp, \
                tc.tile_pool(name="ps_y", bufs=3, space="PSUM") as ps_y:
            inv_o = op_pool.tile([P, NTOK], F32, tag="invo")
            nc.vector.tensor_scalar(inv_o[:], oam[:], EPS, 1.0 / 127.0,
                                    OP.max, OP.mult)
            orq = oqp.tile([P, NTOK], F32, tag="orq")
            nc.vector.reciprocal_approx_fast(orq[:], inv_o[:])
            # quantize otT in place (bf16 holds the int values exactly)
            for c in range(KC):
                otmp = oqp.tile([P, NTOK], F32, tag="otmp")
                nc.vector.tensor_tensor(otmp[:], otT[:, c, :], orq[:],
                                        op=OP.mult)
                nc.vector.tensor_scalar(otT[:, c, :], otmp[:], MAGIC, -MAGIC,
                                        OP.add, OP.add)
            odq = otT

            syT = smp.tile([P, NTB], F32, tag="syT")
            for tb in range(NTB):
                pt = ps_y.tile([P, P], F32, tag="psy", name=f"pt2{tb}")
                nc.tensor.transpose(pt[:], inv_o[:, tb * P:(tb + 1) * P],
                                    idt[:])
                nc.scalar.copy(syT[:, tb:tb + 1], pt[:, 0:1])
            nc.vector.tensor_scalar(syT[:], syT[:], wmean["wo"][:], None,
                                    OP.mult)

            wob3 = wob[:].rearrange("p (c i) -> p c i", c=IC)
            for tb in range(NTB):
                for oh in range(2):
                    py = ps_y.tile([P, DIM // 2], F32, tag="psy",
                                   name=f"py{tb}_{oh}")
                    for c in range(IC):
                        nc.tensor.matmul(
                            py[:],
                            odq[:, c, tb * P:(tb + 1) * P],
                            wob3[:, c, oh * (DIM // 2):(oh + 1) * (DIM // 2)],
                            start=(c == 0), stop=(c == IC - 1))
                    ysb = yp.tile([P, DIM // 2], F32, tag="ysb")
                    nc.scalar.mul(ysb[:], py[:], syT[:, tb:tb + 1])
                    hw = DIM // 4
                    for dh in range(2):
                        nc.sync.dma_start(
                            out=y_out.ap()[tb * P:(tb + 1) * P,
                                           oh * (DIM // 2) + dh * hw:
                                           oh * (DIM // 2) + (dh + 1) * hw],
                            in_=ysb[:, dh * hw:(dh + 1) * hw])
    nc.compile()
    return nc


_CACHE = {}


def _get_nc(key, cfg):
    if key not in _CACHE:
        _CACHE[key] = build(cfg)
    return _CACHE[key]


def _shard(x, context, wq, wk, wv, wo, NTOK):
    b = x.shape[0]
    wmaps = {w + "T": np.ascontiguousarray(a.T)
             for w, a in (("wq", wq), ("wk", wk), ("wv", wv), ("wo", wo))}
    wmaps["iden"] = np.eye(128, dtype=np.float32)
    cores_per_b = N_CORES // b
    in_maps = []
    for core in range(N_CORES):
        bi = core // cores_per_b
        t0 = (core % cores_per_b) * NTOK
        in_maps.append(dict(
            xT=np.ascontiguousarray(x[bi, t0:t0 + NTOK, :].T),
            xN=np.ascontiguousarray(x[bi, t0:t0 + NTOK, :]),
            cT=np.ascontiguousarray(context[bi].T),
            cN=np.ascontiguousarray(context[bi]),
            **wmaps))
    return in_maps


def _assemble(results, b, n, dim, NTOK):
    out = np.empty((b, n, dim), dtype=np.float32)
    cores_per_b = N_CORES // b
    for core in range(N_CORES):
        bi = core // cores_per_b
        t0 = (core % cores_per_b) * NTOK
        out[bi, t0:t0 + NTOK, :] = results[core]["y"]
    return out


def run(x, context, wq, wk, wv, wo, trace=False):
    cfg = CFG_FULL
    b, n, dim = x.shape
    NTOK = cfg["NTOK"]
    nc = _get_nc("full", cfg)
    in_maps = _shard(x, context, wq, wk, wv, wo, NTOK)
    res = run_bass_kernel_spmd(nc, in_maps, list(range(N_CORES)), trace=trace)
    return _assemble(res.results, b, n, dim, NTOK), res


def kernel(x, context, wq, wk, wv, wo):
    return run(x, context, wq, wk, wv, wo, trace=False)[0]


if __name__ == "__main__":
    ins = {k: np.random.randn(*s).astype(np.float32) * (0.02 if k[0] == 'w' else 1.0)
           for k, s in [("x", (2, 2048, 1024)), ("context", (2, 2048, 1024)),
                        ("wq", (1024, 1024)), ("wk", (1024, 1024)),
                        ("wv", (1024, 1024)), ("wo", (1024, 1024))]}
    y = kernel(**ins)
    print("kernel output", y.shape, y.dtype, np.abs(y).max())


# revision 13
# speedup vs baseline: 1.3160x; 1.3160x over previous
"""Trainium2 Bass kernel for BitNet-style cross-attention (8 NeuronCores).

Data-parallel token sharding: b=2, n=2048 -> 4096 query-token rows; each of
the 8 cores owns 512 (cores 0-3 batch 0, 4-7 batch 1) and computes its output
slice independently (k/v recomputed per core).

v3: streamed attention.  The kernel runs in four overlapped phases:
  A/B: x quant + Q proj; ctx half 0 quant + K/V proj; wq/wv/wk quant.
  C:   attention over ctx half 0 (scores+exp+attn@v, po accumulated in
       PSUM then parked unnormalized in SBUF), with ctx-half-1 quant +
       K/V projection emitted as PE/DVE "filler" between attention slabs
       so the Act-engine exp stream (the largest fixed cost) hides under
       projection work.
  D:   attention over ctx half 1 (po += half-1, then per-head softmax
       normalize straight from PSUM), with wo quant as filler.
  tail: out act-quant + output projection.

Quant path: per-token absmax comes from a natural-layout (token-major) copy
of x/ctx (contiguous free-axis reduce) so per-token scales are [P,1] columns
(icT/esc/vsc need no transposes).  Scale rows are broadcast across
partitions once per 512-token group (tiny PE transpose MMs + one gpsimd
partition_broadcast) and the feature-major round chain is 2 fused DVE ops
per eighth via stride-0 broadcast APs.  round() uses the fp32
magic-constant trick.  Softmax denominators accumulate via an extra ones
column in v; their reciprocals use the fast approx DVE reciprocal.
"""

import numpy as np

import concourse.bass as bass
import concourse.mybir as mybir
import concourse.tile as tile
from concourse import bacc, bass_isa
from concourse.bass_utils import run_bass_kernel_spmd

F32 = mybir.dt.float32
BF16 = mybir.dt.bfloat16
AX = mybir.AxisListType
OP = mybir.AluOpType
AF = mybir.ActivationFunctionType

P = 128
MAGIC = 12582912.0  # 1.5 * 2**23: fp32 add/sub rounds to nearest int (ties even)

CFG_FULL = dict(DIM=1024, INNER=1024, H=16, D=64, NTOK=512, MCTX=2048)
N_CORES = 8
EPS = 1e-5
APPROX_DENS = False  # approx reciprocal for softmax denominators


def build(cfg):
    DIM, INNER, H, D = cfg["DIM"], cfg["INNER"], cfg["H"], cfg["D"]
    NTOK, MCTX = cfg["NTOK"], cfg["MCTX"]
    KC = DIM // P            # input-dim 128-chunks (8)
    IC = INNER // P          # inner-dim 128-chunks (8)
    NKB = MCTX // P          # ctx 128-blocks (16)
    NTB = NTOK // P          # query-token 128-blocks (4)
    QTOK = 512               # K-proj moving width
    ETOK = 256               # ctx staging eighth size
    NE = MCTX // ETOK        # 8 eighths
    EKB = ETOK // P          # ctx 128-blocks per eighth (2)
    HKB = NKB // 2           # ctx 128-blocks per half (8)
    VW = D + 1               # v columns per head incl ones
    HPH = (INNER // 2) // D  # heads per inner half (8)
    NP = H // 2              # head pairs (8)

    nc = bacc.Bacc("TRN2", target_bir_lowering=False, debug=False,
                   num_devices=N_CORES)

    xT = nc.dram_tensor("xT", [DIM, NTOK], F32, kind="ExternalInput")
    xN = nc.dram_tensor("xN", [NTOK, DIM], F32, kind="ExternalInput")
    cT = nc.dram_tensor("cT", [DIM, MCTX], F32, kind="ExternalInput")
    cN = nc.dram_tensor("cN", [MCTX, DIM], F32, kind="ExternalInput")
    wT = {}
    for w in ("wq", "wk", "wv", "wo"):
        wT[w] = nc.dram_tensor(w + "T", [DIM, INNER], F32, kind="ExternalInput")
    iden = nc.dram_tensor("iden", [P, P], F32, kind="ExternalInput")
    y_out = nc.dram_tensor("y", [NTOK, DIM], F32, kind="ExternalOutput")

    from contextlib import ExitStack
    with tile.TileContext(nc) as tc, ExitStack() as ctx:
        # ---- long-lived pools -------------------------------------------
        pp = ctx.enter_context(tc.tile_pool(name="persist", bufs=1))
        smp = ctx.enter_context(tc.tile_pool(name="small", bufs=1))
        asp = ctx.enter_context(tc.tile_pool(name="astage", bufs=2))
        # PSUM: ps_mm [*,512] 1-bank tiles (pq/pk/pv/bcast/py) = 2 banks;
        # ps_ss (scores) 2 banks + ps_po 4 banks opened for the attention
        # phases; ps_y for the tail after those close.  Max live = 8 banks.
        ps_mm = ctx.enter_context(tc.tile_pool(name="ps_mm", bufs=2,
                                               space="PSUM"))

        qb = pp.tile([P, IC, NTOK], BF16, tag="qb")     # q*inv_x, feat-major
        kb = pp.tile([P, IC, MCTX], BF16, tag="kb")     # k raw ints, feat-major
        vb = pp.tile([P, NKB * H * VW], BF16, tag="vb")  # v natural + ones col
        vb3 = vb[:].rearrange("p (k h w) -> p k h w", h=H, w=VW)
        idt = pp.tile([P, P], F32, tag="idt")           # identity for PE transp
        nc.sync.dma_start(out=idt[:], in_=iden.ap()[:, :])
        icT = pp.tile([P, NKB], F32, tag="icT")         # inv_c, ctx-token-major
        rqcT = pp.tile([P, NKB], F32, tag="rqcT")       # 127/absmax_c tok-major
        vsc = pp.tile([P, NKB], F32, tag="vsc")         # icT * mean|wv|
        esc = pp.tile([P, NKB], F32, tag="esc")         # icT * mq*mk/sqrt(D)

        wmean = {}

        # ---- weight quantization ----------------------------------------
        def quant_weight(w, wsp, dst_pool, tern_eng="act"):
            NST = 2
            HCH = KC // NST
            HW = HCH * INNER
            wbt = dst_pool.tile([P, KC * INNER], BF16, tag="wb_" + w,
                                name="wb_" + w)
            wsums = smp.tile([P, NST], F32, tag="wsum2")
            stages = []
            for hh in range(NST):
                ws = wsp.tile([P, HW], F32, tag="wst", name=f"ws_{w}{hh}")
                for c in range(HCH):
                    cc = hh * HCH + c
                    nc.sync.dma_start(
                        out=ws[:, c * INNER:(c + 1) * INNER],
                        in_=wT[w].ap()[cc * P:(cc + 1) * P, :])
                nc.scalar.activation(wbt[:, hh * HW:(hh + 1) * HW], ws[:],
                                     AF.Abs, accum_out=wsums[:, hh:hh + 1])
                stages.append(ws)
            wsum = smp.tile([P, 1], F32, tag="wsum")
            nc.vector.tensor_reduce(wsum[:], wsums[:], axis=AX.X, op=OP.add)
            wrep = smp.tile([P, 1], F32, tag="wrep")
            nc.gpsimd.partition_all_reduce(wrep[:], wsum[:], channels=P,
                                           reduce_op=bass_isa.ReduceOp.add)
            mean = smp.tile([P, 1], F32, tag="wmean_" + w, name="mean_" + w)
            nc.vector.tensor_scalar(mean[:], wrep[:], 1.0 / (DIM * INNER),
                                    EPS, OP.mult, OP.max)
            qs = smp.tile([P, 1], F32, tag="wqs_" + w, name="qs_" + w)
            nc.vector.reciprocal(qs[:], mean[:])
            wmean[w] = mean
            for hh, ws in enumerate(stages):
                nc.vector.tensor_scalar(ws[:], ws[:], qs[:], 1.49,
                                        OP.mult, OP.min)
                nc.vector.tensor_scalar(ws[:], ws[:], -1.49, MAGIC,
                                        OP.max, OP.add)
                if tern_eng == "act":
                    nc.scalar.activation(wbt[:, hh * HW:(hh + 1) * HW],
                                         ws[:], AF.Copy, bias=-MAGIC)
                else:
                    nc.vector.tensor_scalar(wbt[:, hh * HW:(hh + 1) * HW],
                                            ws[:], -MAGIC, None, OP.add)
            return wbt

        # wo variant: quarter-staged with re-DMA (small SBUF footprint, runs
        # as filler during attention half 1).  Emitted in units so it can be
        # interleaved between attention slabs.
        def quant_weight_wo_units(wsp, dst_pool):
            w = "wo"
            NWQ = 4
            CPQ = KC // NWQ
            WQW = CPQ * INNER
            wbt = dst_pool.tile([P, KC * INNER], BF16, tag="wb_wo",
                                name="wb_wo")
            wsums = smp.tile([P, NWQ], F32, tag="wsums")
            units = []

            def u_a(i):
                ws = wsp.tile([P, WQW], F32, tag="wstq", name=f"woA{i}")
                for c in range(CPQ):
                    cc = i * CPQ + c
                    nc.sync.dma_start(out=ws[:, c * INNER:(c + 1) * INNER],
                                      in_=wT[w].ap()[cc * P:(cc + 1) * P, :])
                nc.vector.tensor_reduce(wsums[:, i:i + 1], ws[:],
                                        axis=AX.X, op=OP.add,
                                        apply_absolute_value=True)

            def u_mid():
                wsum = smp.tile([P, 1], F32, tag="wsum")
                nc.vector.tensor_reduce(wsum[:], wsums[:], axis=AX.X,
                                        op=OP.add)
                wrep = smp.tile([P, 1], F32, tag="wrep")
                nc.gpsimd.partition_all_reduce(
                    wrep[:], wsum[:], channels=P,
                    reduce_op=bass_isa.ReduceOp.add)
                mean = smp.tile([P, 1], F32, tag="wmean_wo", name="mean_wo")
                nc.vector.tensor_scalar(mean[:], wrep[:],
                                        1.0 / (DIM * INNER), EPS,
                                        OP.mult, OP.max)
                qs = smp.tile([P, 1], F32, tag="wqs_wo", name="qs_wo")
                nc.vector.reciprocal(qs[:], mean[:])
                wmean[w] = mean
                smp_ref["qs_wo"] = qs

            def u_b(i):
                qs = smp_ref["qs_wo"]
                ws = wsp.tile([P, WQW], F32, tag="wstq", name=f"woB{i}")
                for c in range(CPQ):
                    cc = i * CPQ + c
                    nc.sync.dma_start(out=ws[:, c * INNER:(c + 1) * INNER],
                                      in_=wT[w].ap()[cc * P:(cc + 1) * P, :])
                nc.vector.tensor_scalar(ws[:], ws[:], qs[:], 1.49,
                                        OP.mult, OP.min)
                nc.vector.tensor_scalar(ws[:], ws[:], -1.49, MAGIC,
                                        OP.max, OP.add)
                nc.vector.tensor_scalar(wbt[:, i * WQW:(i + 1) * WQW],
                                        ws[:], -MAGIC, None, OP.add)

            smp_ref = {}
            for i in range(NWQ):
                units.append(lambda i=i: u_a(i))
            units.append(u_mid)
            for i in range(NWQ):
                units.append(lambda i=i: u_b(i))
            return wbt, units

        # ---- token-major scale machinery --------------------------------
        def blk_amax(src_dram, blk, inv_col, rq_col, name):
            bn = cnp.tile([P, DIM], F32, tag="bn", name=name)
            nc.sync.dma_start(out=bn[:],
                              in_=src_dram.ap()[blk * P:(blk + 1) * P, :])
            am = smp.tile([P, 1], F32, tag="bam", name="am_" + name)
            nc.vector.tensor_reduce(am[:], bn[:], axis=AX.X, op=OP.max,
                                    apply_absolute_value=True)
            nc.vector.tensor_scalar(inv_col, am[:], EPS, 1.0 / 127.0,
                                    OP.max, OP.mult)
            nc.vector.reciprocal(rq_col, inv_col)

        # transpose nb [P,1] scale columns into one [1, nb*128] row via tiny
        # PE matmuls, evict, broadcast across partitions on gpsimd.
        def bcast_cols(cols_ap, nb, name):
            pt = ps_mm.tile([1, nb * P], F32, tag="psmm", name="bc_" + name)
            for b in range(nb):
                nc.tensor.matmul(pt[0:1, b * P:(b + 1) * P],
                                 cols_ap[:, b:b + 1], idt[:],
                                 start=True, stop=True)
            row = asp.tile([1, nb * P], F32, tag="bcrow", name="bcr_" + name)
            nc.vector.tensor_copy(row[:], pt[:])
            big = asp.tile([P, nb * P], F32, tag="bcbig", name="bcb_" + name)
            nc.gpsimd.partition_broadcast(big[:], row[:])
            return big

        # fused round chain: dst = round(src * rq) over [P, KC, ntok]
        def round_chunks(dst_ap, src_ap, rq_big_ap, ntok):
            rqb = rq_big_ap.unsqueeze(1).to_broadcast((P, KC, ntok))
            nc.vector.tensor_tensor(src_ap, src_ap, rqb, op=OP.mult)
            nc.vector.tensor_scalar(dst_ap, src_ap, MAGIC, -MAGIC,
                                    OP.add, OP.add)

        def ctx_dma(e):
            cs = csp.tile([P, KC, ETOK], F32, tag="cs", name=f"cs{e}")
            col0 = e * ETOK
            for c in range(KC):
                nc.sync.dma_start(
                    out=cs[:, c, :],
                    in_=cT.ap()[c * P:(c + 1) * P, col0:col0 + ETOK])
            return cs

        def ctx_amax(e):
            for b in range(EKB):
                kbk = e * EKB + b
                blk_amax(cN, kbk, icT[:, kbk:kbk + 1],
                         rqcT[:, kbk:kbk + 1], f"cn{kbk}")

        def scales_for_eighth(e, qkm):
            sl = slice(e * EKB, (e + 1) * EKB)
            nc.vector.tensor_scalar(vsc[:, sl], icT[:, sl],
                                    wmean["wv"][:], None, OP.mult)
            nc.vector.tensor_scalar(esc[:, sl], icT[:, sl], qkm[:], None,
                                    OP.mult)

        def ctx_round(e, cs, cdq, rq_big):
            lcol = (e % (NE // 2)) * ETOK
            half = (e % 2) * ETOK
            round_chunks(cdq[:, :, lcol:lcol + ETOK], cs[:],
                         rq_big[:, half:half + ETOK], ETOK)

        # one K-proj chunk: project `ics` inner chunks for q-quarter q
        def k_proj(q, wkb3, cdq, ics, evict):
            lcol = (q % 2) * QTOK
            for ic in ics:
                pk = ps_mm.tile([P, QTOK], F32, tag="psmm",
                                name=f"pk{q}_{ic}")
                for c in range(KC):
                    nc.tensor.matmul(
                        pk[:], wkb3[:, c, ic * P:(ic + 1) * P],
                        cdq[:, c, lcol:lcol + QTOK],
                        start=(c == 0), stop=(c == KC - 1))
                dst = kb[:, ic, q * QTOK:(q + 1) * QTOK]
                if evict == "act":
                    nc.scalar.copy(dst, pk[:])
                else:
                    nc.vector.tensor_copy(dst, pk[:])

        # one V-proj chunk: ctx block kbk, inner half ih
        def v_proj(kbk, ih, wvb3, cdq, evict):
            kk = kbk % HKB
            pv = ps_mm.tile([P, INNER // 2], F32, tag="psmm",
                            name=f"pv{kbk}_{ih}")
            for c in range(KC):
                nc.tensor.matmul(
                    pv[:],
                    cdq[:, c, kk * P:(kk + 1) * P],
                    wvb3[:, c, ih * (INNER // 2):(ih + 1) * (INNER // 2)],
                    start=(c == 0), stop=(c == KC - 1))
            dst = vb3[:, kbk, ih * HPH:(ih + 1) * HPH, 0:D]
            src = pv[:].rearrange("p (h d) -> p h d", d=D)
            if evict == "act":
                nc.scalar.mul(dst, src, vsc[:, kbk:kbk + 1])
            else:
                nc.vector.tensor_scalar(dst, src, vsc[:, kbk:kbk + 1],
                                        None, OP.mult)

        # ---- attention --------------------------------------------------
        def attn_half(h, fillers, state):
            fi = iter(fillers)

            def fill():
                try:
                    f = next(fi)
                except StopIteration:
                    return
                if f is not None:
                    f()

            def ss_mm(hp, kbk, pA, pB):
                ss = ps_ss.tile([P, 2, NTOK], F32, tag="ss")
                for j, ph in enumerate((pA, pB)):
                    nc.tensor.matmul(
                        ss[:, j, :],
                        kb[ph:ph + D, hp, kbk * P:(kbk + 1) * P],
                        qb[ph:ph + D, hp, :],
                        start=True, stop=True)
                return ss

            pos = [(2 * hp, 2 * hp + 1, (2 * hp * D) % P,
                    ((2 * hp + 1) * D) % P) for hp in range(NP)]
            ss_cur = ss_mm(0, h * HKB, pos[0][2], pos[0][3])
            for hp in range(NP):
                hA, hB, pA, pB = pos[hp]
                po = [ps_po.tile([VW, NTOK], F32, tag="po",
                                 name=f"po{h}_{hp}_{j}") for j in range(2)]
                for i in range(HKB):
                    kbk = h * HKB + i
                    et = ep.tile([P, 2, NTOK], BF16, tag="et")
                    nc.scalar.activation(et[:], ss_cur[:], AF.Exp,
                                         scale=esc[:, kbk:kbk + 1])
                    fill()
                    # prefetch next slab's scores while exp runs
                    if i + 1 < HKB:
                        ss_cur = ss_mm(hp, kbk + 1, pA, pB)
                    elif hp + 1 < NP:
                        ss_cur = ss_mm(hp + 1, h * HKB,
                                       pos[hp + 1][2], pos[hp + 1][3])
                    for j, hh in enumerate((hA, hB)):
                        nc.tensor.matmul(
                            po[j][0:VW, :],
                            vb3[:, kbk, hh, :],
                            et[:, j, :],
                            start=(i == 0), stop=(i == HKB - 1))
                # evict po immediately (frees the 2 PSUM banks for the
                # next pair; all remaining work runs from SBUF copies)
                if h == 0:
                    for j in range(2):
                        nc.vector.tensor_copy(otU[0:VW, 2 * hp + j, :],
                                              po[j][:])
                else:
                    dens = rbs.tile([1, 2, NTOK], F32, tag="dens",
                                    name=f"dens{hp}")
                    accs = []
                    for j in range(2):
                        acc = rbp.tile([VW, NTOK], F32, tag=f"acc{j}",
                                       name=f"acc{hp}_{j}")
                        nc.vector.scalar_tensor_tensor(
                            acc[:], po[j][:], 0.0,
                            otU[0:VW, 2 * hp + j, :], OP.add, OP.add)
                        nc.vector.tensor_copy(dens[0:1, j, :],
                                              acc[D:D + 1, :])
                        accs.append(acc)
                    rdp = rbs.tile([1, 2, NTOK], F32, tag="rdp",
                                   name=f"rdp{hp}")
                    if APPROX_DENS:
                        nc.vector.reciprocal_approx_fast(rdp[:], dens[:])
                    else:
                        nc.vector.reciprocal(rdp[:], dens[:])
                    for j, (hh, ph) in enumerate([(hA, pA), (hB, pB)]):
                        rb = rbs.tile([D, NTOK], F32, tag="rb")
                        nc.gpsimd.partition_broadcast(rb[:], rdp[0:1, j, :])
                        nc.vector.tensor_tensor(otT[ph:ph + D, hp, :],
                                                accs[j][0:D, :], rb[:],
                                                op=OP.mult)
                    orep = rbs.tile([P, NTOK], F32, tag="orep",
                                    name=f"orep{hp}")
                    nc.gpsimd.partition_all_reduce(
                        orep[:], otT[:, hp, :], channels=P,
                        reduce_op=bass_isa.ReduceOp.absmax)
                    if hp == 0:
                        nc.vector.tensor_copy(oam[:], orep[:])
                    else:
                        nc.vector.tensor_tensor(oam[:], oam[:], orep[:],
                                                op=OP.max)
            # drain leftover fillers
            for f in fi:
                if f is not None:
                    f()

        # ================= emission =====================================
        # Left-stack pool lifetimes: csp/cnp (staging, close end of C),
        # cq0 (ctx half-0 ints, closes end of B), wstage (closes end of B,
        # opened after cq0), xstage/xq/wbqq (A only), cq1 (C only).
        csp_cm = tc.tile_pool(name="cstage", bufs=2)
        csp = csp_cm.__enter__()
        cnp_cm = tc.tile_pool(name="cnstage", bufs=2)
        cnp = cnp_cm.__enter__()
        cq0_cm = tc.tile_pool(name="cq0", bufs=1)
        cq0p = cq0_cm.__enter__()
        cdq0 = cq0p.tile([P, KC, MCTX // 2], BF16, tag="cdq0")
        wsp_cm = tc.tile_pool(name="wstage", bufs=2)
        wsp = wsp_cm.__enter__()

        # ---------- phase A: x + wq + Q proj ----------
        with tc.tile_pool(name="xq", bufs=1) as xqp:
            with tc.tile_pool(name="xstage", bufs=1) as xsp:
                xs = xsp.tile([P, KC, NTOK], F32, tag="xs")
                for c in range(KC):
                    nc.sync.dma_start(
                        out=xs[:, c, :],
                        in_=xT.ap()[c * P:(c + 1) * P, :])
                invx4 = smp.tile([P, NTB], F32, tag="invx4")
                rqx4 = smp.tile([P, NTB], F32, tag="rqx4")
                for b in range(NTB):
                    blk_amax(xN, b, invx4[:, b:b + 1], rqx4[:, b:b + 1],
                             f"xn{b}")
                cs0 = ctx_dma(0)
                ctx_amax(0)
                cs1 = ctx_dma(1)
                ctx_amax(1)
                rqx_b = bcast_cols(rqx4[:], NTB, "rqx")
                invx_b = bcast_cols(invx4[:], NTB, "invx")
                xdq = xqp.tile([P, KC, NTOK], BF16, tag="xdq")
                round_chunks(xdq[:, :, 0:ETOK], xs[:, :, 0:ETOK],
                             rqx_b[:, 0:ETOK], ETOK)
                round_chunks(xdq[:, :, ETOK:NTOK], xs[:, :, ETOK:NTOK],
                             rqx_b[:, ETOK:NTOK], ETOK)

            with tc.tile_pool(name="wbqq", bufs=1) as wbpq:
                wqb = quant_weight("wq", wsp, wbpq, tern_eng="act")
                rqc_b0 = bcast_cols(rqcT[:, 0:NTB], NTB, "rqc0")
                ctx_round(0, cs0, cdq0, rqc_b0)
                ctx_round(1, cs1, cdq0, rqc_b0)
                wqb3 = wqb[:].rearrange("p (c i) -> p c i", c=KC)
                for ic in range(IC):
                    pq = ps_mm.tile([P, NTOK], F32, tag="psmm",
                                    name=f"pq{ic}")
                    for c in range(KC):
                        nc.tensor.matmul(
                            pq[:], wqb3[:, c, ic * P:(ic + 1) * P],
                            xdq[:, c, :],
                            start=(c == 0), stop=(c == KC - 1))
                    nc.vector.tensor_tensor(qb[:, ic, :], pq[:],
                                            invx_b[:], op=OP.mult)

        # ---------- phase B: ctx half 0 + wv/wk + K/V proj half 0 --------
        cs2 = ctx_dma(2)
        ctx_amax(2)
        cs3 = ctx_dma(3)
        ctx_amax(3)
        rqc_b1 = bcast_cols(rqcT[:, NTB:2 * NTB], NTB, "rqc1")
        ctx_round(2, cs2, cdq0, rqc_b1)
        ctx_round(3, cs3, cdq0, rqc_b1)
        wbpk = ctx.enter_context(
            tc.tile_pool(name="wbqk", bufs=1, side="right"))
        wkb = quant_weight("wk", wsp, wbpk, tern_eng="act")
        wkb3 = wkb[:].rearrange("p (c i) -> p c i", c=KC)

        qkm = smp.tile([P, 1], F32, tag="qkm")
        nc.vector.tensor_tensor(qkm[:], wmean["wq"][:], wmean["wk"][:],
                                op=OP.mult)
        nc.vector.tensor_scalar(qkm[:], qkm[:], 1.0 / float(np.sqrt(D)),
                                None, OP.mult)
        k_proj(0, wkb3, cdq0, range(IC), evict="dve")
        k_proj(1, wkb3, cdq0, range(IC), evict="dve")

        wbpv = ctx.enter_context(
            tc.tile_pool(name="wbqv", bufs=1, side="right"))
        wvb = quant_weight("wv", wsp, wbpv, tern_eng="act")
        wvb3 = wvb[:].rearrange("p (c i) -> p c i", c=KC)
        for e in range(4):
            scales_for_eighth(e, qkm)
        nc.vector.memset(vb3[:, :, :, D], 1.0)
        for kbk in range(HKB):
            for ih in range(2):
                v_proj(kbk, ih, wvb3, cdq0, evict="act")

        wsp_cm.__exit__(None, None, None)
        cq0_cm.__exit__(None, None, None)

        # ---------- phase C: attention half 0 | ctx half 1 + K/V ---------
        cq1_cm = tc.tile_pool(name="cq1", bufs=1)
        cq1p = cq1_cm.__enter__()
        cdq1 = cq1p.tile([P, KC, MCTX // 2], BF16, tag="cdq1")

        otup = ctx.enter_context(tc.tile_pool(name="otup", bufs=1,
                                              side="right"))
        otU = otup.tile([VW, H, NTOK], BF16, tag="otU")
        ep = ctx.enter_context(tc.tile_pool(name="etile", bufs=2,
                                            side="right"))
        rbp = ctx.enter_context(tc.tile_pool(name="rbpool", bufs=2,
                                             side="right"))
        rbs = ctx.enter_context(tc.tile_pool(name="rbsing", bufs=1,
                                             side="right"))
        op_pool = ctx.enter_context(tc.tile_pool(name="opool", bufs=1,
                                                 side="right"))
        otT = op_pool.tile([P, IC, NTOK], BF16, tag="otT")
        oam = op_pool.tile([P, NTOK], F32, tag="oam")

        ps_ss_cm = tc.tile_pool(name="ps_ss", bufs=2, space="PSUM")
        ps_ss = ps_ss_cm.__enter__()
        ps_po_cm = tc.tile_pool(name="ps_po", bufs=2, space="PSUM")
        ps_po = ps_po_cm.__enter__()

        st = {}

        def u_cdma(e):
            st[e] = ctx_dma(e)

        def u_amax(e):
            ctx_amax(e)
            scales_for_eighth(e, qkm)

        def u_bcast(q):
            st["rq%d" % q] = bcast_cols(rqcT[:, q * NTB:(q + 1) * NTB],
                                        NTB, f"rqc{q}")

        def u_round(e):
            ctx_round(e, st.pop(e), cdq1, st["rq%d" % (e // 2)])

        fillers = []

        def F(fn, *a):
            fillers.append(lambda fn=fn, a=a: fn(*a))

        F(u_cdma, 4)
        F(u_amax, 4)
        F(u_cdma, 5)
        F(u_amax, 5)
        F(u_bcast, 2)
        F(u_round, 4)
        F(u_round, 5)
        for ic0 in range(0, IC, 2):
            F(k_proj, 2, wkb3, cdq1, [ic0, ic0 + 1], "dve")
        F(u_cdma, 6)
        F(u_amax, 6)
        F(u_cdma, 7)
        F(u_amax, 7)
        F(u_bcast, 3)
        F(u_round, 6)
        F(u_round, 7)
        for ic0 in range(0, IC, 2):
            F(k_proj, 3, wkb3, cdq1, [ic0, ic0 + 1], "dve")
        for kbk in range(HKB, NKB):
            for ih in range(2):
                F(v_proj, kbk, ih, wvb3, cdq1, "dve")

        attn_half(0, fillers, st)

        cq1_cm.__exit__(None, None, None)
        cnp_cm.__exit__(None, None, None)
        csp_cm.__exit__(None, None, None)

        # ---------- phase D: attention half 1 | wo quant -----------------
        wop = ctx.enter_context(tc.tile_pool(name="wopool", bufs=1,
                                             side="right"))
        wsp2_cm = tc.tile_pool(name="wstage2", bufs=2)
        wsp2 = wsp2_cm.__enter__()
        wob, wo_units = quant_weight_wo_units(wsp2, wop)

        attn_half(1, wo_units, st)

        wsp2_cm.__exit__(None, None, None)
        ps_po_cm.__exit__(None, None, None)
        ps_ss_cm.__exit__(None, None, None)

        # ---------- tail: attn-out quantization + output projection ------
        with tc.tile_pool(name="oq", bufs=2) as oqp, \
                tc.tile_pool(name="ysb", bufs=4) as yp, \
                tc.tile_pool(name="ps_y", bufs=3, space="PSUM") as ps_y:
            inv_o = op_pool.tile([P, NTOK], F32, tag="invo")
            nc.vector.tensor_scalar(inv_o[:], oam[:], EPS, 1.0 / 127.0,
                                    OP.max, OP.mult)
            orq = oqp.tile([P, NTOK], F32, tag="orq")
            nc.vector.reciprocal_approx_fast(orq[:], inv_o[:])
            # quantize otT in place (bf16 holds the int values exactly)
            for c in range(KC):
                otmp = oqp.tile([P, NTOK], F32, tag="otmp")
                nc.vector.tensor_tensor(otmp[:], otT[:, c, :], orq[:],
                                        op=OP.mult)
                nc.vector.tensor_scalar(otT[:, c, :], otmp[:], MAGIC, -MAGIC,
                                        OP.add, OP.add)
            odq = otT

            syT = smp.tile([P, NTB], F32, tag="syT")
            for tb in range(NTB):
                pt = ps_y.tile([P, P], F32, tag="psy", name=f"pt2{tb}")
                nc.tensor.transpose(pt[:], inv_o[:, tb * P:(tb + 1) * P],
                                    idt[:])
                nc.scalar.copy(syT[:, tb:tb + 1], pt[:, 0:1])
            nc.vector.tensor_scalar(syT[:], syT[:], wmean["wo"][:], None,
                                    OP.mult)

            wob3 = wob[:].rearrange("p (c i) -> p c i", c=IC)
            for tb in range(NTB):
                for oh in range(2):
                    py = ps_y.tile([P, DIM // 2], F32, tag="psy",
                                   name=f"py{tb}_{oh}")
                    for c in range(IC):
                        nc.tensor.matmul(
                            py[:],
                            odq[:, c, tb * P:(tb + 1) * P],
                            wob3[:, c, oh * (DIM // 2):(oh + 1) * (DIM // 2)],
                            start=(c == 0), stop=(c == IC - 1))
                    ysb = yp.tile([P, DIM // 2], F32, tag="ysb")
                    nc.scalar.mul(ysb[:], py[:], syT[:, tb:tb + 1])
                    hw = DIM // 4
                    for dh in range(2):
                        nc.sync.dma_start(
                            out=y_out.ap()[tb * P:(tb + 1) * P,
                                           oh * (DIM // 2) + dh * hw:
                                           oh * (DIM // 2) + (dh + 1) * hw],
                            in_=ysb[:, dh * hw:(dh + 1) * hw])
    nc.compile()
    return nc


_CACHE = {}


def _get_nc(key, cfg):
    if key not in _CACHE:
        _CACHE[key] = build(cfg)
    return _CACHE[key]


def _shard(x, context, wq, wk, wv, wo, NTOK):
    b = x.shape[0]
    wmaps = {w + "T": np.ascontiguousarray(a.T)
             for w, a in (("wq", wq), ("wk", wk), ("wv", wv), ("wo", wo))}
    wmaps["iden"] = np.eye(128, dtype=np.float32)
    cores_per_b = N_CORES // b
    in_maps = []
    for core in range(N_CORES):
        bi = core // cores_per_b
        t0 = (core % cores_per_b) * NTOK
        in_maps.append(dict(
            xT=np.ascontiguousarray(x[bi, t0:t0 + NTOK, :].T),
            xN=np.ascontiguousarray(x[bi, t0:t0 + NTOK, :]),
            cT=np.ascontiguousarray(context[bi].T),
            cN=np.ascontiguousarray(context[bi]),
            **wmaps))
    return in_maps


def _assemble(results, b, n, dim, NTOK):
    out = np.empty((b, n, dim), dtype=np.float32)
    cores_per_b = N_CORES // b
    for core in range(N_CORES):
        bi = core // cores_per_b
        t0 = (core % cores_per_b) * NTOK
        out[bi, t0:t0 + NTOK, :] = results[core]["y"]
    return out


def run(x, context, wq, wk, wv, wo, trace=False):
    cfg = CFG_FULL
    b, n, dim = x.shape
    NTOK = cfg["NTOK"]
    nc = _get_nc("full", cfg)
    in_maps = _shard(x, context, wq, wk, wv, wo, NTOK)
    res = run_bass_kernel_spmd(nc, in_maps, list(range(N_CORES)), trace=trace)
    return _assemble(res.results, b, n, dim, NTOK), res


def kernel(x, context, wq, wk, wv, wo):
    return run(x, context, wq, wk, wv, wo, trace=False)[0]


if __name__ == "__main__":
    ins = {k: np.random.randn(*s).astype(np.float32) * (0.02 if k[0] == 'w' else 1.0)
           for k, s in [("x", (2, 2048, 1024)), ("context", (2, 2048, 1024)),
                        ("wq", (1024, 1024)), ("wk", (1024, 1024)),
                        ("wv", (1024, 1024)), ("wo", (1024, 1024))]}
    y = kernel(**ins)
    print("kernel output", y.shape, y.dtype, np.abs(y).max())


# revision 16
# speedup vs baseline: 1.5661x; 1.1900x over previous
"""Trainium2 Bass kernel for BitNet-style cross-attention (8 NeuronCores).

Data-parallel token sharding: b=2, n=2048 -> 4096 query-token rows; each of
the 8 cores owns 512 (cores 0-3 batch 0, 4-7 batch 1) and computes its output
slice independently (k/v recomputed per core).

v3: streamed attention.  The kernel runs in four overlapped phases:
  A/B: x quant + Q proj; ctx half 0 quant + K/V proj; wq/wv/wk quant.
  C:   attention over ctx half 0 (scores+exp+attn@v, po accumulated in
       PSUM then parked unnormalized in SBUF), with ctx-half-1 quant +
       K/V projection emitted as PE/DVE "filler" between attention slabs
       so the Act-engine exp stream (the largest fixed cost) hides under
       projection work.
  D:   attention over ctx half 1 (po += half-1, then per-head softmax
       normalize straight from PSUM), with wo quant as filler.
  tail: out act-quant + output projection.

Quant path: per-token absmax comes from a natural-layout (token-major) copy
of x/ctx (contiguous free-axis reduce) so per-token scales are [P,1] columns
(icT/esc/vsc need no transposes).  Scale rows are broadcast across
partitions once per 512-token group (tiny PE transpose MMs + one gpsimd
partition_broadcast) and the feature-major round chain is 2 fused DVE ops
per eighth via stride-0 broadcast APs.  round() uses the fp32
magic-constant trick.  Softmax denominators accumulate via an extra ones
column in v; their reciprocals use the fast approx DVE reciprocal.
"""

import numpy as np

import concourse.bass as bass
import concourse.mybir as mybir
import concourse.tile as tile
from concourse import bacc, bass_isa
from concourse.bass_utils import run_bass_kernel_spmd

F32 = mybir.dt.float32
BF16 = mybir.dt.bfloat16
AX = mybir.AxisListType
OP = mybir.AluOpType
AF = mybir.ActivationFunctionType

P = 128
MAGIC = 12582912.0  # 1.5 * 2**23: fp32 add/sub rounds to nearest int (ties even)

CFG_FULL = dict(DIM=1024, INNER=1024, H=16, D=64, NTOK=512, MCTX=2048)
N_CORES = 8
EPS = 1e-5
APPROX_DENS = False  # approx reciprocal for softmax denominators


def build(cfg):
    DIM, INNER, H, D = cfg["DIM"], cfg["INNER"], cfg["H"], cfg["D"]
    NTOK, MCTX = cfg["NTOK"], cfg["MCTX"]
    KC = DIM // P            # input-dim 128-chunks (8)
    IC = INNER // P          # inner-dim 128-chunks (8)
    NKB = MCTX // P          # ctx 128-blocks (16)
    NTB = NTOK // P          # query-token 128-blocks (4)
    QTOK = 512               # K-proj moving width
    ETOK = 256               # ctx staging eighth size
    NE = MCTX // ETOK        # 8 eighths
    EKB = ETOK // P          # ctx 128-blocks per eighth (2)
    HKB = NKB // 2           # ctx 128-blocks per half (8)
    VW = D + 1               # v columns per head incl ones
    HPH = (INNER // 2) // D  # heads per inner half (8)
    NP = H // 2              # head pairs (8)

    nc = bacc.Bacc("TRN2", target_bir_lowering=False, debug=False,
                   num_devices=N_CORES)

    xT = nc.dram_tensor("xT", [DIM, NTOK], F32, kind="ExternalInput")
    xN = nc.dram_tensor("xN", [NTOK, DIM], F32, kind="ExternalInput")
    cT = nc.dram_tensor("cT", [DIM, MCTX], F32, kind="ExternalInput")
    cN = nc.dram_tensor("cN", [MCTX, DIM], F32, kind="ExternalInput")
    wT = {}
    for w in ("wq", "wk", "wv", "wo"):
        wT[w] = nc.dram_tensor(w + "T", [DIM, INNER], F32, kind="ExternalInput")
    iden = nc.dram_tensor("iden", [P, P], F32, kind="ExternalInput")
    y_out = nc.dram_tensor("y", [NTOK, DIM], F32, kind="ExternalOutput")

    from contextlib import ExitStack
    with tile.TileContext(nc) as tc, ExitStack() as ctx:
        # ---- long-lived pools -------------------------------------------
        pp = ctx.enter_context(tc.tile_pool(name="persist", bufs=1))
        smp = ctx.enter_context(tc.tile_pool(name="small", bufs=1))
        asp = ctx.enter_context(tc.tile_pool(name="astage", bufs=2))
        brp = ctx.enter_context(tc.tile_pool(name="browp", bufs=1))
        # PSUM: ps_mm [*,512] 1-bank tiles (pq/pk/pv/bcast/py) = 2 banks;
        # ps_ss (scores) 2 banks + ps_po 4 banks opened for the attention
        # phases; ps_y for the tail after those close.  Max live = 8 banks.
        ps_mm = ctx.enter_context(tc.tile_pool(name="ps_mm", bufs=2,
                                               space="PSUM"))

        qb = pp.tile([P, IC, NTOK], BF16, tag="qb")     # q*inv_x, feat-major
        kb = pp.tile([P, IC, MCTX], BF16, tag="kb")     # k raw ints, feat-major
        vb = pp.tile([P, NKB * H * VW], BF16, tag="vb")  # v natural + ones col
        vb3 = vb[:].rearrange("p (k h w) -> p k h w", h=H, w=VW)
        idt = pp.tile([P, P], F32, tag="idt")           # identity for PE transp
        nc.sync.dma_start(out=idt[:], in_=iden.ap()[:, :])
        icT = pp.tile([P, NKB], F32, tag="icT")         # inv_c, ctx-token-major
        rqcT = pp.tile([P, NKB], F32, tag="rqcT")       # 127/absmax_c tok-major
        vsc = pp.tile([P, NKB], F32, tag="vsc")         # icT * mean|wv|
        esc = pp.tile([P, NKB], F32, tag="esc")         # icT * mq*mk/sqrt(D)

        wmean = {}

        # ---- weight quantization ----------------------------------------
        def quant_weight(w, wsp, dst_pool, tern_eng="act"):
            NST = 4
            HCH = KC // NST
            HW = HCH * INNER
            wbt = dst_pool.tile([P, KC * INNER], BF16, tag="wb_" + w,
                                name="wb_" + w)
            wsums = smp.tile([P, NST], F32, tag="wsum2")
            stages = []
            for hh in range(NST):
                ws = wsp.tile([P, HW], F32, tag="wst", name=f"ws_{w}{hh}")
                for c in range(HCH):
                    cc = hh * HCH + c
                    nc.sync.dma_start(
                        out=ws[:, c * INNER:(c + 1) * INNER],
                        in_=wT[w].ap()[cc * P:(cc + 1) * P, :])
                nc.scalar.activation(wbt[:, hh * HW:(hh + 1) * HW], ws[:],
                                     AF.Abs, accum_out=wsums[:, hh:hh + 1])
                stages.append(ws)
            wsum = smp.tile([P, 1], F32, tag="wsum")
            nc.vector.tensor_reduce(wsum[:], wsums[:], axis=AX.X, op=OP.add)
            wrep = smp.tile([P, 1], F32, tag="wrep")
            nc.gpsimd.partition_all_reduce(wrep[:], wsum[:], channels=P,
                                           reduce_op=bass_isa.ReduceOp.add)
            mean = smp.tile([P, 1], F32, tag="wmean_" + w, name="mean_" + w)
            nc.vector.tensor_scalar(mean[:], wrep[:], 1.0 / (DIM * INNER),
                                    EPS, OP.mult, OP.max)
            qs = smp.tile([P, 1], F32, tag="wqs_" + w, name="qs_" + w)
            nc.vector.reciprocal(qs[:], mean[:])
            wmean[w] = mean
            for hh, ws in enumerate(stages):
                nc.vector.tensor_scalar(ws[:], ws[:], qs[:], 1.49,
                                        OP.mult, OP.min)
                nc.vector.tensor_scalar(ws[:], ws[:], -1.49, MAGIC,
                                        OP.max, OP.add)
                if tern_eng == "act":
                    nc.scalar.activation(wbt[:, hh * HW:(hh + 1) * HW],
                                         ws[:], AF.Copy, bias=-MAGIC)
                else:
                    nc.vector.tensor_scalar(wbt[:, hh * HW:(hh + 1) * HW],
                                            ws[:], -MAGIC, None, OP.add)
            return wbt

        # wo variant: quarter-staged with re-DMA (small SBUF footprint, runs
        # as filler during attention half 1).  Emitted in units so it can be
        # interleaved between attention slabs.
        def quant_weight_wo_units(wsp, dst_pool):
            w = "wo"
            NWQ = 4
            CPQ = KC // NWQ
            WQW = CPQ * INNER
            wbt = dst_pool.tile([P, KC * INNER], BF16, tag="wb_wo",
                                name="wb_wo")
            wsums = smp.tile([P, NWQ], F32, tag="wsums")
            units = []

            def u_a(i):
                ws = wsp.tile([P, WQW], F32, tag="wstq", name=f"woA{i}")
                for c in range(CPQ):
                    cc = i * CPQ + c
                    nc.sync.dma_start(out=ws[:, c * INNER:(c + 1) * INNER],
                                      in_=wT[w].ap()[cc * P:(cc + 1) * P, :])
                nc.vector.tensor_reduce(wsums[:, i:i + 1], ws[:],
                                        axis=AX.X, op=OP.add,
                                        apply_absolute_value=True)

            def u_mid():
                wsum = smp.tile([P, 1], F32, tag="wsum")
                nc.vector.tensor_reduce(wsum[:], wsums[:], axis=AX.X,
                                        op=OP.add)
                wrep = smp.tile([P, 1], F32, tag="wrep")
                nc.gpsimd.partition_all_reduce(
                    wrep[:], wsum[:], channels=P,
                    reduce_op=bass_isa.ReduceOp.add)
                mean = smp.tile([P, 1], F32, tag="wmean_wo", name="mean_wo")
                nc.vector.tensor_scalar(mean[:], wrep[:],
                                        1.0 / (DIM * INNER), EPS,
                                        OP.mult, OP.max)
                qs = smp.tile([P, 1], F32, tag="wqs_wo", name="qs_wo")
                nc.vector.reciprocal(qs[:], mean[:])
                wmean[w] = mean
                smp_ref["qs_wo"] = qs

            def u_b(i):
                qs = smp_ref["qs_wo"]
                ws = wsp.tile([P, WQW], F32, tag="wstq", name=f"woB{i}")
                for c in range(CPQ):
                    cc = i * CPQ + c
                    nc.sync.dma_start(out=ws[:, c * INNER:(c + 1) * INNER],
                                      in_=wT[w].ap()[cc * P:(cc + 1) * P, :])
                nc.vector.tensor_scalar(ws[:], ws[:], qs[:], 1.49,
                                        OP.mult, OP.min)
                nc.vector.tensor_scalar(ws[:], ws[:], -1.49, MAGIC,
                                        OP.max, OP.add)
                nc.vector.tensor_scalar(wbt[:, i * WQW:(i + 1) * WQW],
                                        ws[:], -MAGIC, None, OP.add)

            smp_ref = {}
            for i in range(NWQ):
                units.append(lambda i=i: u_a(i))
            units.append(u_mid)
            for i in range(NWQ):
                units.append(lambda i=i: u_b(i))
            return wbt, units

        # ---- token-major scale machinery --------------------------------
        def blk_amax(src_dram, blk, inv_col, rq_col, name):
            bn = cnp.tile([P, DIM], F32, tag="bn", name=name)
            nc.sync.dma_start(out=bn[:],
                              in_=src_dram.ap()[blk * P:(blk + 1) * P, :])
            am = smp.tile([P, 1], F32, tag="bam", name="am_" + name)
            nc.vector.tensor_reduce(am[:], bn[:], axis=AX.X, op=OP.max,
                                    apply_absolute_value=True)
            nc.vector.tensor_scalar(inv_col, am[:], EPS, 1.0 / 127.0,
                                    OP.max, OP.mult)
            nc.vector.reciprocal(rq_col, inv_col)

        # transpose nb [P,1] scale columns into one [1, nb*128] row via tiny
        # PE matmuls, evict, broadcast across partitions on gpsimd.
        def bcast_cols(cols_ap, nb, name):
            pt = ps_mm.tile([1, nb * P], F32, tag="psmm", name="bc_" + name)
            for b in range(nb):
                nc.tensor.matmul(pt[0:1, b * P:(b + 1) * P],
                                 cols_ap[:, b:b + 1], idt[:],
                                 start=True, stop=True)
            row = brp.tile([1, nb * P], F32, tag="bcrow", name="bcr_" + name)
            nc.vector.tensor_copy(row[:], pt[:])
            big = asp.tile([P, nb * P], F32, tag="bcbig", name="bcb_" + name)
            nc.gpsimd.partition_broadcast(big[:], row[:])
            return big

        # fused round chain: dst = round(src * rq) over [P, KC, ntok]
        def round_chunks(dst_ap, src_ap, rq_big_ap, ntok):
            rqb = rq_big_ap.unsqueeze(1).to_broadcast((P, KC, ntok))
            nc.vector.tensor_tensor(src_ap, src_ap, rqb, op=OP.mult)
            nc.vector.tensor_scalar(dst_ap, src_ap, MAGIC, -MAGIC,
                                    OP.add, OP.add)

        def ctx_dma(e):
            cs = csp.tile([P, KC, ETOK], F32, tag="cs", name=f"cs{e}")
            col0 = e * ETOK
            for c in range(KC):
                nc.sync.dma_start(
                    out=cs[:, c, :],
                    in_=cT.ap()[c * P:(c + 1) * P, col0:col0 + ETOK])
            return cs

        def ctx_amax(e):
            for b in range(EKB):
                kbk = e * EKB + b
                blk_amax(cN, kbk, icT[:, kbk:kbk + 1],
                         rqcT[:, kbk:kbk + 1], f"cn{kbk}")

        def scales_for_eighth(e, qkm):
            sl = slice(e * EKB, (e + 1) * EKB)
            nc.vector.tensor_scalar(vsc[:, sl], icT[:, sl],
                                    wmean["wv"][:], None, OP.mult)
            nc.vector.tensor_scalar(esc[:, sl], icT[:, sl], qkm[:], None,
                                    OP.mult)

        def ctx_round(e, cs, cdq, rq_big):
            lcol = (e % (NE // 2)) * ETOK
            half = (e % 2) * ETOK
            round_chunks(cdq[:, :, lcol:lcol + ETOK], cs[:],
                         rq_big[:, half:half + ETOK], ETOK)

        # one K-proj chunk: project `ics` inner chunks for q-quarter q
        def k_proj(q, wkb3, cdq, ics, evict):
            lcol = (q % 2) * QTOK
            for ic in ics:
                pk = ps_mm.tile([P, QTOK], F32, tag="psmm",
                                name=f"pk{q}_{ic}")
                for c in range(KC):
                    nc.tensor.matmul(
                        pk[:], wkb3[:, c, ic * P:(ic + 1) * P],
                        cdq[:, c, lcol:lcol + QTOK],
                        start=(c == 0), stop=(c == KC - 1))
                dst = kb[:, ic, q * QTOK:(q + 1) * QTOK]
                if evict == "act":
                    nc.scalar.copy(dst, pk[:])
                else:
                    nc.vector.tensor_copy(dst, pk[:])

        # one V-proj chunk: ctx block kbk, inner half ih
        def v_proj(kbk, ih, wvb3, cdq, evict):
            kk = kbk % HKB
            pv = ps_mm.tile([P, INNER // 2], F32, tag="psmm",
                            name=f"pv{kbk}_{ih}")
            for c in range(KC):
                nc.tensor.matmul(
                    pv[:],
                    cdq[:, c, kk * P:(kk + 1) * P],
                    wvb3[:, c, ih * (INNER // 2):(ih + 1) * (INNER // 2)],
                    start=(c == 0), stop=(c == KC - 1))
            dst = vb3[:, kbk, ih * HPH:(ih + 1) * HPH, 0:D]
            src = pv[:].rearrange("p (h d) -> p h d", d=D)
            if evict == "act":
                nc.scalar.mul(dst, src, vsc[:, kbk:kbk + 1])
            else:
                nc.vector.tensor_scalar(dst, src, vsc[:, kbk:kbk + 1],
                                        None, OP.mult)

        # ---- attention --------------------------------------------------
        def attn_half(h, fillers, state):
            fi = iter(fillers)
            pend = []

            def fill():
                try:
                    f = next(fi)
                except StopIteration:
                    return
                if f is not None:
                    f()

            def ss_mm(hp, kbk, pA, pB):
                ss = ps_ss.tile([P, 2, NTOK], F32, tag="ss")
                for j, ph in enumerate((pA, pB)):
                    nc.tensor.matmul(
                        ss[:, j, :],
                        kb[ph:ph + D, hp, kbk * P:(kbk + 1) * P],
                        qb[ph:ph + D, hp, :],
                        start=True, stop=True)
                return ss

            pos = [(2 * hp, 2 * hp + 1, (2 * hp * D) % P,
                    ((2 * hp + 1) * D) % P) for hp in range(NP)]
            ss_cur = ss_mm(0, h * HKB, pos[0][2], pos[0][3])
            for hp in range(NP):
                hA, hB, pA, pB = pos[hp]
                po = [ps_po.tile([VW, NTOK], F32, tag="po",
                                 name=f"po{h}_{hp}_{j}") for j in range(2)]
                for i in range(HKB):
                    kbk = h * HKB + i
                    et = ep.tile([P, 2, NTOK], BF16, tag="et")
                    nc.scalar.activation(et[:], ss_cur[:], AF.Exp,
                                         scale=esc[:, kbk:kbk + 1])
                    fill()
                    # prefetch next slab's scores while exp runs
                    if i + 1 < HKB:
                        ss_cur = ss_mm(hp, kbk + 1, pA, pB)
                    elif hp + 1 < NP:
                        ss_cur = ss_mm(hp + 1, h * HKB,
                                       pos[hp + 1][2], pos[hp + 1][3])
                    for j, hh in enumerate((hA, hB)):
                        nc.tensor.matmul(
                            po[j][0:VW, :],
                            vb3[:, kbk, hh, :],
                            et[:, j, :],
                            start=(i == 0), stop=(i == HKB - 1))
                # evict po immediately (frees the 2 PSUM banks for the
                # next pair; all remaining work runs from SBUF copies)
                if h == 0:
                    for j in range(2):
                        nc.vector.tensor_copy(otU[0:VW, 2 * hp + j, :],
                                              po[j][:])
                else:
                    # evict FIRST (frees po banks for the next pair); the
                    # reciprocal/normalize for this pair is deferred until
                    # after the next pair's eviction so it never blocks
                    # the PSUM hand-off on the in-order DVE queue.
                    # alternate slots: this pair's dens must not wait on the
                    # previous pair's (still-pending) reciprocal
                    dens = rbs.tile([1, 2, NTOK], F32, tag=f"dens{hp % 2}",
                                    name=f"dens{hp}")
                    accs = []
                    for j in range(2):
                        acc = rbp.tile([VW, NTOK], F32, tag=f"acc{j}",
                                       name=f"acc{hp}_{j}")
                        nc.vector.scalar_tensor_tensor(
                            acc[:], po[j][:], 0.0,
                            otU[0:VW, 2 * hp + j, :], OP.add, OP.add)
                        nc.vector.tensor_copy(dens[0:1, j, :],
                                              acc[D:D + 1, :])
                        accs.append(acc)
                    pend.append((hp, pos[hp], dens, accs))
                    if len(pend) > 1 or hp == NP - 1:
                        todo = pend[:-1] if hp < NP - 1 else pend
                        for (qp, (qA, qB, qpA, qpB), qdens, qaccs) in todo:
                            rdp = rbs.tile([1, 2, NTOK], F32, tag="rdp",
                                           name=f"rdp{qp}")
                            if APPROX_DENS:
                                nc.vector.reciprocal_approx_fast(
                                    rdp[:], qdens[:])
                            else:
                                nc.vector.reciprocal(rdp[:], qdens[:])
                            for j, ph in enumerate((qpA, qpB)):
                                rb = rbs.tile([D, NTOK], F32, tag="rb")
                                nc.gpsimd.partition_broadcast(
                                    rb[:], rdp[0:1, j, :])
                                nc.vector.tensor_tensor(
                                    otT[ph:ph + D, qp, :],
                                    qaccs[j][0:D, :], rb[:], op=OP.mult)
                            orep = rbs.tile([P, NTOK], F32, tag="orep",
                                            name=f"orep{qp}")
                            nc.gpsimd.partition_all_reduce(
                                orep[:], otT[:, qp, :], channels=P,
                                reduce_op=bass_isa.ReduceOp.absmax)
                            if qp == 0:
                                nc.vector.tensor_copy(oam[:], orep[:])
                            else:
                                nc.vector.tensor_tensor(
                                    oam[:], oam[:], orep[:], op=OP.max)
                        pend[:] = [] if hp == NP - 1 else pend[-1:]
            # drain leftover fillers
            for f in fi:
                if f is not None:
                    f()

        # ================= emission =====================================
        # Left-stack pool lifetimes: csp/cnp (staging, close end of C),
        # cq0 (ctx half-0 ints, closes end of B), wstage (closes end of B,
        # opened after cq0), xstage/xq/wbqq (A only), cq1 (C only).
        csp_cm = tc.tile_pool(name="cstage", bufs=2)
        csp = csp_cm.__enter__()
        cnp_cm = tc.tile_pool(name="cnstage", bufs=2)
        cnp = cnp_cm.__enter__()
        cq0_cm = tc.tile_pool(name="cq0", bufs=1)
        cq0p = cq0_cm.__enter__()
        cdq0 = cq0p.tile([P, KC, MCTX // 2], BF16, tag="cdq0")
        wsp_cm = tc.tile_pool(name="wstage", bufs=4)
        wsp = wsp_cm.__enter__()

        # ---------- phase A: x + wq + Q proj ----------
        with tc.tile_pool(name="xq", bufs=1) as xqp:
            with tc.tile_pool(name="xstage", bufs=1) as xsp:
                xs = xsp.tile([P, KC, NTOK], F32, tag="xs")
                for c in range(KC):
                    nc.sync.dma_start(
                        out=xs[:, c, :],
                        in_=xT.ap()[c * P:(c + 1) * P, :])
                invx4 = smp.tile([P, NTB], F32, tag="invx4")
                rqx4 = smp.tile([P, NTB], F32, tag="rqx4")
                for b in range(NTB):
                    blk_amax(xN, b, invx4[:, b:b + 1], rqx4[:, b:b + 1],
                             f"xn{b}")
                cs0 = ctx_dma(0)
                ctx_amax(0)
                cs1 = ctx_dma(1)
                ctx_amax(1)
                rqx_b = bcast_cols(rqx4[:], NTB, "rqx")
                invx_b = bcast_cols(invx4[:], NTB, "invx")
                xdq = xqp.tile([P, KC, NTOK], BF16, tag="xdq")
                round_chunks(xdq[:, :, 0:ETOK], xs[:, :, 0:ETOK],
                             rqx_b[:, 0:ETOK], ETOK)
                round_chunks(xdq[:, :, ETOK:NTOK], xs[:, :, ETOK:NTOK],
                             rqx_b[:, ETOK:NTOK], ETOK)

            with tc.tile_pool(name="wbqq", bufs=1) as wbpq:
                wqb = quant_weight("wq", wsp, wbpq, tern_eng="act")
                rqc_b0 = bcast_cols(rqcT[:, 0:NTB], NTB, "rqc0")
                ctx_round(0, cs0, cdq0, rqc_b0)
                ctx_round(1, cs1, cdq0, rqc_b0)
                wqb3 = wqb[:].rearrange("p (c i) -> p c i", c=KC)
                for ic in range(IC):
                    pq = ps_mm.tile([P, NTOK], F32, tag="psmm",
                                    name=f"pq{ic}")
                    for c in range(KC):
                        nc.tensor.matmul(
                            pq[:], wqb3[:, c, ic * P:(ic + 1) * P],
                            xdq[:, c, :],
                            start=(c == 0), stop=(c == KC - 1))
                    nc.vector.tensor_tensor(qb[:, ic, :], pq[:],
                                            invx_b[:], op=OP.mult)

        # ---------- phase B: ctx half 0 + wv/wk + K/V proj half 0 --------
        cs2 = ctx_dma(2)
        ctx_amax(2)
        cs3 = ctx_dma(3)
        ctx_amax(3)
        rqc_b1 = bcast_cols(rqcT[:, NTB:2 * NTB], NTB, "rqc1")
        ctx_round(2, cs2, cdq0, rqc_b1)
        ctx_round(3, cs3, cdq0, rqc_b1)
        wbpk = ctx.enter_context(
            tc.tile_pool(name="wbqk", bufs=1, side="right"))
        wkb = quant_weight("wk", wsp, wbpk, tern_eng="act")
        wkb3 = wkb[:].rearrange("p (c i) -> p c i", c=KC)

        qkm = smp.tile([P, 1], F32, tag="qkm")
        nc.vector.tensor_tensor(qkm[:], wmean["wq"][:], wmean["wk"][:],
                                op=OP.mult)
        nc.vector.tensor_scalar(qkm[:], qkm[:], 1.0 / float(np.sqrt(D)),
                                None, OP.mult)
        k_proj(0, wkb3, cdq0, range(IC), evict="dve")
        k_proj(1, wkb3, cdq0, range(IC), evict="dve")

        wbpv = ctx.enter_context(
            tc.tile_pool(name="wbqv", bufs=1, side="right"))
        wvb = quant_weight("wv", wsp, wbpv, tern_eng="act")
        wvb3 = wvb[:].rearrange("p (c i) -> p c i", c=KC)
        for e in range(4):
            scales_for_eighth(e, qkm)
        nc.vector.memset(vb3[:, :, :, D], 1.0)
        for kbk in range(HKB):
            for ih in range(2):
                v_proj(kbk, ih, wvb3, cdq0, evict="act")

        wsp_cm.__exit__(None, None, None)
        cq0_cm.__exit__(None, None, None)

        # ---------- phase C: attention half 0 | ctx half 1 + K/V ---------
        cq1_cm = tc.tile_pool(name="cq1", bufs=1)
        cq1p = cq1_cm.__enter__()
        cdq1 = cq1p.tile([P, KC, MCTX // 2], BF16, tag="cdq1")

        otup = ctx.enter_context(tc.tile_pool(name="otup", bufs=1,
                                              side="right"))
        otU = otup.tile([VW, H, NTOK], BF16, tag="otU")
        ep = ctx.enter_context(tc.tile_pool(name="etile", bufs=2,
                                            side="right"))
        rbp = ctx.enter_context(tc.tile_pool(name="rbpool", bufs=2,
                                             side="right"))
        rbs = ctx.enter_context(tc.tile_pool(name="rbsing", bufs=1,
                                             side="right"))
        op_pool = ctx.enter_context(tc.tile_pool(name="opool", bufs=1,
                                                 side="right"))
        otT = op_pool.tile([P, IC, NTOK], BF16, tag="otT")
        oam = op_pool.tile([P, NTOK], F32, tag="oam")

        ps_ss_cm = tc.tile_pool(name="ps_ss", bufs=2, space="PSUM")
        ps_ss = ps_ss_cm.__enter__()
        ps_po_cm = tc.tile_pool(name="ps_po", bufs=2, space="PSUM")
        ps_po = ps_po_cm.__enter__()

        st = {}

        def u_cdma(e):
            st[e] = ctx_dma(e)

        def u_amax(e):
            ctx_amax(e)
            scales_for_eighth(e, qkm)

        def u_bcast(q):
            st["rq%d" % q] = bcast_cols(rqcT[:, q * NTB:(q + 1) * NTB],
                                        NTB, f"rqc{q}")

        def u_round(e):
            ctx_round(e, st.pop(e), cdq1, st["rq%d" % (e // 2)])

        fillers = []

        def F(fn, *a):
            fillers.append(lambda fn=fn, a=a: fn(*a))

        F(u_cdma, 4)
        F(u_amax, 4)
        F(u_cdma, 5)
        F(u_amax, 5)
        F(u_bcast, 2)
        F(u_round, 4)
        F(u_round, 5)
        for ic0 in range(0, IC, 2):
            F(k_proj, 2, wkb3, cdq1, [ic0, ic0 + 1], "dve")
        F(u_cdma, 6)
        F(u_amax, 6)
        F(u_cdma, 7)
        F(u_amax, 7)
        F(u_bcast, 3)
        F(u_round, 6)
        F(u_round, 7)
        for ic0 in range(0, IC, 2):
            F(k_proj, 3, wkb3, cdq1, [ic0, ic0 + 1], "dve")
        for kbk in range(HKB, NKB):
            for ih in range(2):
                F(v_proj, kbk, ih, wvb3, cdq1, "dve")

        attn_half(0, fillers, st)

        cq1_cm.__exit__(None, None, None)
        cnp_cm.__exit__(None, None, None)
        csp_cm.__exit__(None, None, None)

        # ---------- phase D: attention half 1 | wo quant -----------------
        wop = ctx.enter_context(tc.tile_pool(name="wopool", bufs=1,
                                             side="right"))
        wsp2_cm = tc.tile_pool(name="wstage2", bufs=2)
        wsp2 = wsp2_cm.__enter__()
        wob, wo_units = quant_weight_wo_units(wsp2, wop)

        attn_half(1, wo_units, st)

        wsp2_cm.__exit__(None, None, None)
        ps_po_cm.__exit__(None, None, None)
        ps_ss_cm.__exit__(None, None, None)

        # ---------- tail: attn-out quantization + output projection ------
        with tc.tile_pool(name="oq", bufs=2) as oqp, \
                tc.tile_pool(name="ysb", bufs=4) as yp, \
                tc.tile_pool(name="ps_y", bufs=3, space="PSUM") as ps_y:
            inv_o = op_pool.tile([P, NTOK], F32, tag="invo")
            nc.vector.tensor_scalar(inv_o[:], oam[:], EPS, 1.0 / 127.0,
                                    OP.max, OP.mult)
            orq = oqp.tile([P, NTOK], F32, tag="orq")
            nc.vector.reciprocal_approx_fast(orq[:], inv_o[:])
            # quantize otT in place (bf16 holds the int values exactly)
            for c in range(KC):
                otmp = oqp.tile([P, NTOK], F32, tag="otmp")
                nc.vector.tensor_tensor(otmp[:], otT[:, c, :], orq[:],
                                        op=OP.mult)
                nc.vector.tensor_scalar(otT[:, c, :], otmp[:], MAGIC, -MAGIC,
                                        OP.add, OP.add)
            odq = otT

            syT = smp.tile([P, NTB], F32, tag="syT")
            for tb in range(NTB):
                pt = ps_y.tile([P, P], F32, tag="psy", name=f"pt2{tb}")
                nc.tensor.transpose(pt[:], inv_o[:, tb * P:(tb + 1) * P],
                                    idt[:])
                nc.scalar.copy(syT[:, tb:tb + 1], pt[:, 0:1])
            nc.vector.tensor_scalar(syT[:], syT[:], wmean["wo"][:], None,
                                    OP.mult)

            wob3 = wob[:].rearrange("p (c i) -> p c i", c=IC)
            for tb in range(NTB):
                for oh in range(2):
                    py = ps_y.tile([P, DIM // 2], F32, tag="psy",
                                   name=f"py{tb}_{oh}")
                    for c in range(IC):
                        nc.tensor.matmul(
                            py[:],
                            odq[:, c, tb * P:(tb + 1) * P],
                            wob3[:, c, oh * (DIM // 2):(oh + 1) * (DIM // 2)],
                            start=(c == 0), stop=(c == IC - 1))
                    ysb = yp.tile([P, DIM // 2], F32, tag="ysb")
                    nc.scalar.mul(ysb[:], py[:], syT[:, tb:tb + 1])
                    hw = DIM // 4
                    for dh in range(2):
                        nc.sync.dma_start(
                            out=y_out.ap()[tb * P:(tb + 1) * P,
                                           oh * (DIM // 2) + dh * hw:
                                           oh * (DIM // 2) + (dh + 1) * hw],
                            in_=ysb[:, dh * hw:(dh + 1) * hw])
    nc.compile()
    return nc


_CACHE = {}


def _get_nc(key, cfg):
    if key not in _CACHE:
        _CACHE[key] = build(cfg)
    return _CACHE[key]


def _shard(x, context, wq, wk, wv, wo, NTOK):
    b = x.shape[0]
    wmaps = {w + "T": np.ascontiguousarray(a.T)
             for w, a in (("wq", wq), ("wk", wk), ("wv", wv), ("wo", wo))}
    wmaps["iden"] = np.eye(128, dtype=np.float32)
    cores_per_b = N_CORES // b
    in_maps = []
    for core in range(N_CORES):
        bi = core // cores_per_b
        t0 = (core % cores_per_b) * NTOK
        in_maps.append(dict(
            xT=np.ascontiguousarray(x[bi, t0:t0 + NTOK, :].T),
            xN=np.ascontiguousarray(x[bi, t0:t0 + NTOK, :]),
            cT=np.ascontiguousarray(context[bi].T),
            cN=np.ascontiguousarray(context[bi]),
            **wmaps))
    return in_maps


def _assemble(results, b, n, dim, NTOK):
    out = np.empty((b, n, dim), dtype=np.float32)
    cores_per_b = N_CORES // b
    for core in range(N_CORES):
        bi = core // cores_per_b
        t0 = (core % cores_per_b) * NTOK
        out[bi, t0:t0 + NTOK, :] = results[core]["y"]
    return out


def run(x, context, wq, wk, wv, wo, trace=False):
    cfg = CFG_FULL
    b, n, dim = x.shape
    NTOK = cfg["NTOK"]
    nc = _get_nc("full", cfg)
    in_maps = _shard(x, context, wq, wk, wv, wo, NTOK)
    res = run_bass_kernel_spmd(nc, in_maps, list(range(N_CORES)), trace=trace)
    return _assemble(res.results, b, n, dim, NTOK), res


def kernel(x, context, wq, wk, wv, wo):
    return run(x, context, wq, wk, wv, wo, trace=False)[0]


if __name__ == "__main__":
    ins = {k: np.random.randn(*s).astype(np.float32) * (0.02 if k[0] == 'w' else 1.0)
           for k, s in [("x", (2, 2048, 1024)), ("context", (2, 2048, 1024)),
                        ("wq", (1024, 1024)), ("wk", (1024, 1024)),
                        ("wv", (1024, 1024)), ("wo", (1024, 1024))]}
    y = kernel(**ins)
    print("kernel output", y.shape, y.dtype, np.abs(y).max())
